# revision 22
# baseline (speedup 1.0000x reference)
"""Trainium2 Bass kernel for nn_ConditionalAttentionLayer.

Row-sharded across 8 NeuronCores: core c computes output rows
[c*512, (c+1)*512).  Math identities used on device:

    exp(leaky_relu(s)) = max(exp(s), exp(0.2*s)),  s = e_src[i] + e_dst[j]

factors rank-1, and softmax is scale-invariant per column, so the v_i =
exp(e_src[i]) factor cancels:  the device builds

    P[j,i] = adjT[j,i] * max(u_j, r_i * p_j)

with u = exp(e_dst)*c, p = exp(0.2*e_dst)*c, r = exp(-0.8*e_src), c a
per-mech scale keeping every value < 1 so the adjacency mask is a plain
min().  Per j-block this costs one 4x-mode tensor_scalar (mult,max) per
mechanism on DVE plus one masking tensor_tensor split between DVE and
Pool.  Mechanism 3 instead uses max(u,rp) = u + relu(rp-u): ACT computes
relu(p*r - u) and the u*adjT term folds into an extra PE matmul against
host-precomputed hu3 = h_aug3*u3, spreading the elementwise work over
all four engines.  adj is transposed/cast to bf16 on the host so the
device streams it straight into SBUF tiles.
"""

import sys
from contextlib import ExitStack

import numpy as np
import ml_dtypes

sys.path.insert(0, "/opt/trn_rl_repo")

import concourse.bass as bass  # noqa: E402
import concourse.bacc as bacc  # noqa: E402
import concourse.tile as tile  # noqa: E402
import concourse.mybir as mybir  # noqa: E402
from concourse import bass_utils  # noqa: E402
from concourse.masks import make_identity  # noqa: E402

N = 4096
INS = 256
OUTS = 64
M = 4
NCORES = 8
ROWS = N // NCORES      # 512 output rows per core
JB = N // 128           # 32 j-blocks
IT = ROWS // 128        # 4 i-tiles per core
SD = 256                # mech-0 pass-1 columns on DVE; Pool takes the rest
POOL_P1 = True          # debug: route Pool pass-1 work to DVE when False
ACT_P1 = True           # debug: emulate ACT relu path on DVE when False
LEAK = 0.2
K_MARGIN = 0.95         # keep max(u, r*p) < 1 so min(x, adjT) masks

F32 = mybir.dt.float32
BF16 = mybir.dt.bfloat16
Alu = mybir.AluOpType
Act = mybir.ActivationFunctionType


def _trace_kernel(tc, out_d, adjt_d, h_d, hu_d, rb_d, pu_d):
    nc = tc.nc
    with ExitStack() as ctx:
        const = ctx.enter_context(tc.tile_pool(name="const", bufs=1))
        kp = ctx.enter_context(tc.tile_pool(name="kp", bufs=4))
        pp = ctx.enter_context(tc.tile_pool(name="pp", bufs=4))
        accp = ctx.enter_context(tc.tile_pool(name="acc", bufs=1, space="PSUM"))
        tpp = ctx.enter_context(tc.tile_pool(name="tp", bufs=3, space="PSUM"))
        fin = ctx.enter_context(tc.tile_pool(name="fin", bufs=8))

        # ---- persistent SBUF tensors ----
        pu_sb = const.tile([128, M, 3, JB], F32, tag="pu")
        rb_sb = const.tile([128, M, ROWS], BF16, tag="rb")
        h_g = [const.tile([128, 4, M, 65], BF16, tag=f"h{g}", name=f"h{g}")
               for g in range(8)]
        hu_g = [[const.tile([128, 4, 65], BF16, tag=f"hu{m}a", name=f"hu{m}a"),
                 const.tile([128, 28, 65], BF16, tag=f"hu{m}b", name=f"hu{m}b")]
                for m in (2, 3)]
        adjt_g = [const.tile([128, 2, ROWS], BF16, tag=f"adjt{g}",
                             name=f"adjt{g}") for g in range(16)]
        ident = const.tile([128, 128], F32, tag="ident")
        make_identity(nc, ident)
        actw = const.tile([128, 1], F32, tag="actw")
        nc.scalar.activation(actw, ident[:, 0:1], Act.Relu)
        warm = accp.tile([128, 128], F32, tag="warm", name="warm")
        for _ in range(25):
            nc.tensor.transpose(warm, ident, ident)

        # small params first, then chunks ordered so jb 0 unblocks fast.
        # adjt/h/hu chunk tiles are whole-tile DMA targets: partial-slice
        # writes into one big tile lose WAR/RAW deps in the tile framework.
        nc.sync.dma_start(rb_sb, rb_d)
        nc.sync.dma_start(pu_sb, pu_d)
        nc.sync.dma_start(adjt_g[0], adjt_d[:, 0:2, :])
        nc.sync.dma_start(h_g[0], h_d[:, 0:4, :, :])
        nc.sync.dma_start(hu_g[0][0], hu_d[0][:, 0:4, :])
        nc.sync.dma_start(hu_g[1][0], hu_d[1][:, 0:4, :])
        nc.sync.dma_start(adjt_g[1], adjt_d[:, 2:4, :])
        nc.sync.dma_start(hu_g[0][1], hu_d[0][:, 4:32, :])
        nc.sync.dma_start(hu_g[1][1], hu_d[1][:, 4:32, :])
        for g in range(2, 16):
            a, b = g * 2, (g + 1) * 2
            nc.sync.dma_start(adjt_g[g], adjt_d[:, a:b, :])
            if g % 2 == 1:
                gg = (g - 1) // 2
                nc.sync.dma_start(h_g[gg], h_d[:, gg * 4:(gg + 1) * 4, :, :])

        # ---- psum accumulators: one [65, ROWS] bank per mechanism ----
        acc = [accp.tile([65, ROWS], F32, tag=f"acc{m}", name=f"acc{m}")
               for m in range(M)]

        # ---- heavy loop ----
        for jb in range(JB):
            k_t = kp.tile([128, M, ROWS], BF16, tag="k")
            # mech 0: K = (r * p_j) max u_j, split DVE / Pool by columns
            nc.vector.tensor_scalar(
                k_t[:, 0, 0:SD], rb_sb[:, 0, 0:SD],
                pu_sb[:, 0, 0, jb:jb + 1], pu_sb[:, 0, 1, jb:jb + 1],
                Alu.mult, op1=Alu.max,
            )
            eng0 = nc.gpsimd if POOL_P1 else nc.vector
            eng0.tensor_scalar(
                k_t[:, 0, SD:ROWS], rb_sb[:, 0, SD:ROWS],
                pu_sb[:, 0, 0, jb:jb + 1], pu_sb[:, 0, 1, jb:jb + 1],
                Alu.mult, op1=Alu.max,
            )
            # mech 1 fully on Pool
            eng0.tensor_scalar(
                k_t[:, 1, :], rb_sb[:, 1, :],
                pu_sb[:, 1, 0, jb:jb + 1], pu_sb[:, 1, 1, jb:jb + 1],
                Alu.mult, op1=Alu.max,
            )
            # mechs 2,3 on ACT: K' = relu(r*p_j - u_j); u_j*adjT goes via PE
            for m in (2, 3):
                if ACT_P1:
                    nc.scalar.activation(
                        k_t[:, m, :], rb_sb[:, m, :], Act.Relu,
                        bias=pu_sb[:, m, 2, jb:jb + 1],
                        scale=pu_sb[:, m, 0, jb:jb + 1],
                    )
                else:
                    nc.vector.tensor_scalar(
                        k_t[:, m, :], rb_sb[:, m, :],
                        pu_sb[:, m, 0, jb:jb + 1], pu_sb[:, m, 2, jb:jb + 1],
                        Alu.mult, op1=Alu.add,
                    )
                    nc.vector.tensor_scalar(
                        k_t[:, m, :], k_t[:, m, :], 0.0, None, Alu.max,
                    )
            # mask: P = K min adjT (adjT broadcast across mechs) on DVE
            p_t = pp.tile([128, M, ROWS], BF16, tag="p")
            at = adjt_g[jb // 2][:, jb % 2, :]
            at_b = bass.AP(at.tensor, at.offset,
                           [list(at.ap[0]), [0, M], [1, ROWS]])
            nc.vector.tensor_tensor(p_t, k_t, at_b, Alu.min)
            # accumulate out^T[m] += h_aug[jb, m].T @ P[m]
            for m in range(M):
                nc.tensor.matmul(
                    acc[m],
                    lhsT=h_g[jb // 4][:, jb % 4, m, :],
                    rhs=p_t[:, m, :],
                    start=(jb == 0), stop=(jb == JB - 1),
                )
            # mech 2,3 u-terms: acc_m += hu_m[jb].T @ adjT[jb]
            for i, m in enumerate((2, 3)):
                hu_l = (hu_g[i][0][:, jb, :] if jb < 4
                        else hu_g[i][1][:, jb - 4, :])
                nc.tensor.matmul(
                    acc[m],
                    lhsT=hu_l,
                    rhs=at,
                    start=False, stop=(jb == JB - 1),
                )

        # ---- epilogue: transpose, normalize, elu, store ----
        # per 128-row chunk: 4 transposes into one PSUM tile, one strided
        # reciprocal over the 4 denominators, then ELU on the whole chunk.
        out_r = out_d.rearrange("(c p) f -> c p f", p=128)
        o65d = {}
        for cp in range(IT // 2):
            for m in range(M):
                o65 = fin.tile([65, 256], F32, tag="o65", name="o65")
                nc.scalar.activation(
                    o65, acc[m][:, cp * 256:(cp + 1) * 256], Act.Copy)
                o65d[(cp, m)] = o65
        for c in range(IT):
            pt_t = tpp.tile([128, M, 65], F32, tag="ptt")
            for m in range(M):
                o65 = o65d[(c // 2, m)]
                h = (c % 2) * 128
                nc.tensor.transpose(pt_t[:, m, :], o65[:, h:h + 128],
                                    ident[0:65, 0:65])
            rcp = fin.tile([128, M], F32, tag="rcp")
            pt_dn = pt_t[:, 0, 64:65]
            dn = bass.AP(pt_dn.tensor, pt_dn.offset,
                         [list(pt_dn.ap[0]), [65, M]])
            nc.vector.reciprocal(rcp, dn)
            xn = fin.tile([128, M, OUTS], F32, tag="xn")
            for m in range(3):
                nc.vector.tensor_scalar(
                    xn[:, m, :], pt_t[:, m, 0:OUTS], rcp[:, m:m + 1], None,
                    Alu.mult)
            nc.scalar.activation(xn[:, 3, :], pt_t[:, 3, 0:OUTS], Act.Copy,
                                 scale=rcp[:, 3:4])
            ob = fin.tile([128, M * OUTS], F32, tag="ob")
            mn = fin.tile([128, M * OUTS], F32, tag="mn")
            nc.gpsimd.tensor_scalar(mn, xn, 0.0, None, Alu.min)
            eq = fin.tile([128, M * OUTS], F32, tag="eq")
            nc.scalar.activation(eq, mn, Act.Exp)
            nc.vector.scalar_tensor_tensor(
                ob, eq, -1.0, xn.rearrange("p m o -> p (m o)"),
                Alu.add, Alu.max,
            )
            nc.sync.dma_start(out_r[c], ob)


_CACHE = {}


def _build():
    if "nc" in _CACHE:
        return _CACHE["nc"]
    nc = bacc.Bacc("TRN2", target_bir_lowering=False, debug=False,
                   num_devices=NCORES)
    adjt_d = nc.dram_tensor("adjt", [128, JB, ROWS], BF16,
                            kind="ExternalInput").ap()
    h_d = nc.dram_tensor("h_aug", [128, JB, M, 65], BF16,
                         kind="ExternalInput").ap()
    hu_d = [nc.dram_tensor(f"hu{m}", [128, JB, 65], BF16,
                           kind="ExternalInput").ap() for m in (2, 3)]
    rb_d = nc.dram_tensor("rb", [128, M, ROWS], BF16,
                          kind="ExternalInput").ap()
    pu_d = nc.dram_tensor("pu", [128, M, 3, JB], F32,
                          kind="ExternalInput").ap()
    out_d = nc.dram_tensor("out", [ROWS, M * OUTS], F32,
                           kind="ExternalOutput").ap()
    with tile.TileContext(nc) as tc:
        _trace_kernel(tc, out_d, adjt_d, h_d, hu_d, rb_d, pu_d)
    nc.compile()
    _CACHE["nc"] = nc
    return nc


def host_prep(x, adj, W, a1, a2, Wc, bc):
    x = np.asarray(x, np.float32)
    pooled = x.mean(0)
    gb = (pooled @ np.asarray(Wc, np.float32) + np.asarray(bc, np.float32))
    gb = gb.reshape(2, M, OUTS)
    gamma, beta = gb[0], gb[1]
    h = np.einsum("ni,mio->mno", x, np.asarray(W, np.float32))
    h = gamma[:, None, :] * h + beta[:, None, :]          # [M, N, OUTS]
    e_src = np.einsum("mno,mo->mn", h, np.asarray(a1, np.float32))
    e_dst = np.einsum("mno,mo->mn", h, np.asarray(a2, np.float32))

    u = np.exp(e_dst)                    # [M, N]
    p = np.exp(LEAK * e_dst)
    r = np.exp((LEAK - 1.0) * e_src)
    c = K_MARGIN / np.maximum(u.max(axis=1),
                              r.max(axis=1) * p.max(axis=1))  # [M]
    u *= c[:, None]
    p *= c[:, None]

    # h_aug [128, JB, M, 65] lhsT tiles (ones column -> softmax denominator)
    h_aug = np.zeros((N, M, 65), np.float32)
    for m in range(M):
        h_aug[:, m, 0:OUTS] = h[m]
        h_aug[:, m, OUTS] = 1.0
    h_tiles = np.ascontiguousarray(
        h_aug.reshape(JB, 128, M, 65).transpose(1, 0, 2, 3)
    ).astype(ml_dtypes.bfloat16)

    # hu_m [128, JB, 65]: h_aug of mech m scaled by u_m per row j
    hu_tiles = {}
    for m in (2, 3):
        hum = h_aug[:, m, :] * u[m][:, None]
        hu_tiles[m] = np.ascontiguousarray(
            hum.reshape(JB, 128, 65).transpose(1, 0, 2)
        ).astype(ml_dtypes.bfloat16)

    # per-j-block scalar columns: [128, M, 3, JB] (p, u, -u)
    pu = np.empty((128, M, 3, JB), np.float32)
    for m in range(M):
        pu[:, m, 0, :] = p[m].reshape(JB, 128).T
        pu[:, m, 1, :] = u[m].reshape(JB, 128).T
        pu[:, m, 2, :] = -u[m].reshape(JB, 128).T

    adjb = np.asarray(adj, np.int32).astype(ml_dtypes.bfloat16)

    in_maps = []
    for cc in range(NCORES):
        sl = slice(cc * ROWS, (cc + 1) * ROWS)
        adjt = np.ascontiguousarray(
            adjb[sl].T.reshape(JB, 128, ROWS).transpose(1, 0, 2))
        rb = np.ascontiguousarray(
            np.broadcast_to(r[:, sl], (128, M, ROWS))
        ).astype(ml_dtypes.bfloat16)
        in_maps.append({
            "adjt": adjt,
            "h_aug": h_tiles,
            "hu2": hu_tiles[2],
            "hu3": hu_tiles[3],
            "rb": rb,
            "pu": pu,
        })
    return in_maps


def kernel(x, adj, W, a1, a2, Wc, bc):
    nc = _build()
    in_maps = host_prep(x, adj, W, a1, a2, Wc, bc)
    res = bass_utils.run_bass_kernel_spmd(
        nc, in_maps, core_ids=list(range(NCORES))
    )
    out = np.concatenate([res.results[c]["out"] for c in range(NCORES)], axis=0)
    return out.astype(np.float32)


# revision 23
# speedup vs baseline: 1.0074x; 1.0074x over previous
"""Trainium2 Bass kernel for nn_ConditionalAttentionLayer.

Row-sharded across 8 NeuronCores: core c computes output rows
[c*512, (c+1)*512).  Math identities used on device:

    exp(leaky_relu(s)) = max(exp(s), exp(0.2*s)),  s = e_src[i] + e_dst[j]

factors rank-1, and softmax is scale-invariant per column, so the v_i =
exp(e_src[i]) factor cancels:  the device builds

    P[j,i] = adjT[j,i] * max(u_j, r_i * p_j)

with u = exp(e_dst)*c, p = exp(0.2*e_dst)*c, r = exp(-0.8*e_src), c a
per-mech scale keeping every value < 1 so the adjacency mask is a plain
min().  Per j-block this costs one 4x-mode tensor_scalar (mult,max) per
mechanism on DVE plus one masking tensor_tensor split between DVE and
Pool.  Mechanism 3 instead uses max(u,rp) = u + relu(rp-u): ACT computes
relu(p*r - u) and the u*adjT term folds into an extra PE matmul against
host-precomputed hu3 = h_aug3*u3, spreading the elementwise work over
all four engines.  adj is transposed/cast to bf16 on the host so the
device streams it straight into SBUF tiles.
"""

import sys
from contextlib import ExitStack

import numpy as np
import ml_dtypes

sys.path.insert(0, "/opt/trn_rl_repo")

import concourse.bass as bass  # noqa: E402
import concourse.bacc as bacc  # noqa: E402
import concourse.tile as tile  # noqa: E402
import concourse.mybir as mybir  # noqa: E402
from concourse import bass_utils  # noqa: E402
from concourse.masks import make_identity  # noqa: E402

N = 4096
INS = 256
OUTS = 64
M = 4
NCORES = 8
ROWS = N // NCORES      # 512 output rows per core
JB = N // 128           # 32 j-blocks
IT = ROWS // 128        # 4 i-tiles per core
SD = 256                # mech-0 pass-1 columns on DVE; Pool takes the rest
POOL_P1 = True          # debug: route Pool pass-1 work to DVE when False
ACT_P1 = True           # debug: emulate ACT relu path on DVE when False
LEAK = 0.2
K_MARGIN = 0.95         # keep max(u, r*p) < 1 so min(x, adjT) masks

F32 = mybir.dt.float32
BF16 = mybir.dt.bfloat16
Alu = mybir.AluOpType
Act = mybir.ActivationFunctionType


def _trace_kernel(tc, out_d, adjt_d, h_d, hu_d, rb_d, pu_d):
    nc = tc.nc
    with ExitStack() as ctx:
        const = ctx.enter_context(tc.tile_pool(name="const", bufs=1))
        kp = ctx.enter_context(tc.tile_pool(name="kp", bufs=4))
        pp = ctx.enter_context(tc.tile_pool(name="pp", bufs=4))
        accp = ctx.enter_context(tc.tile_pool(name="acc", bufs=1, space="PSUM"))
        tpp = ctx.enter_context(tc.tile_pool(name="tp", bufs=3, space="PSUM"))
        fin = ctx.enter_context(tc.tile_pool(name="fin", bufs=8))

        # ---- persistent SBUF tensors ----
        pu_sb = const.tile([128, M, 3, JB], F32, tag="pu")
        rb_sb = const.tile([128, M, ROWS], BF16, tag="rb")
        h_g = [const.tile([128, 4, M, 65], BF16, tag=f"h{g}", name=f"h{g}")
               for g in range(8)]
        hu_g = [[const.tile([128, 4, 65], BF16, tag=f"hu{m}a", name=f"hu{m}a"),
                 const.tile([128, 28, 65], BF16, tag=f"hu{m}b", name=f"hu{m}b")]
                for m in (2, 3)]
        adjt_g = [const.tile([128, 2, ROWS], BF16, tag=f"adjt{g}",
                             name=f"adjt{g}") for g in range(16)]
        ident = const.tile([128, 128], F32, tag="ident")
        make_identity(nc, ident)
        actw = const.tile([128, 1], F32, tag="actw")
        nc.scalar.activation(actw, ident[:, 0:1], Act.Relu)
        warm = accp.tile([128, 128], F32, tag="warm", name="warm")
        for _ in range(25):
            nc.tensor.transpose(warm, ident, ident)

        # small params first, then chunks ordered so jb 0 unblocks fast.
        # adjt/h/hu chunk tiles are whole-tile DMA targets: partial-slice
        # writes into one big tile lose WAR/RAW deps in the tile framework.
        nc.sync.dma_start(rb_sb, rb_d)
        nc.sync.dma_start(pu_sb, pu_d)
        nc.sync.dma_start(adjt_g[0], adjt_d[:, 0:2, :])
        nc.sync.dma_start(h_g[0], h_d[:, 0:4, :, :])
        nc.sync.dma_start(hu_g[0][0], hu_d[0][:, 0:4, :])
        nc.sync.dma_start(hu_g[1][0], hu_d[1][:, 0:4, :])
        nc.sync.dma_start(adjt_g[1], adjt_d[:, 2:4, :])
        nc.sync.dma_start(hu_g[0][1], hu_d[0][:, 4:32, :])
        nc.sync.dma_start(hu_g[1][1], hu_d[1][:, 4:32, :])
        for g in range(2, 16):
            a, b = g * 2, (g + 1) * 2
            nc.sync.dma_start(adjt_g[g], adjt_d[:, a:b, :])
            if g % 2 == 1:
                gg = (g - 1) // 2
                nc.sync.dma_start(h_g[gg], h_d[:, gg * 4:(gg + 1) * 4, :, :])

        # ---- psum accumulators: one [65, ROWS] bank per mechanism ----
        acc = [accp.tile([65, ROWS], F32, tag=f"acc{m}", name=f"acc{m}")
               for m in range(M)]

        # ---- heavy loop ----
        for jb in range(JB):
            k_t = kp.tile([128, M, ROWS], BF16, tag="k")
            # mech 0: K = (r * p_j) max u_j, split DVE / Pool by columns
            nc.vector.tensor_scalar(
                k_t[:, 0, 0:SD], rb_sb[:, 0, 0:SD],
                pu_sb[:, 0, 0, jb:jb + 1], pu_sb[:, 0, 1, jb:jb + 1],
                Alu.mult, op1=Alu.max,
            )
            eng0 = nc.gpsimd if POOL_P1 else nc.vector
            eng0.tensor_scalar(
                k_t[:, 0, SD:ROWS], rb_sb[:, 0, SD:ROWS],
                pu_sb[:, 0, 0, jb:jb + 1], pu_sb[:, 0, 1, jb:jb + 1],
                Alu.mult, op1=Alu.max,
            )
            # mech 1 fully on Pool
            eng0.tensor_scalar(
                k_t[:, 1, :], rb_sb[:, 1, :],
                pu_sb[:, 1, 0, jb:jb + 1], pu_sb[:, 1, 1, jb:jb + 1],
                Alu.mult, op1=Alu.max,
            )
            # mechs 2,3 on ACT: K' = relu(r*p_j - u_j); u_j*adjT goes via PE
            for m in (2, 3):
                if ACT_P1:
                    nc.scalar.activation(
                        k_t[:, m, :], rb_sb[:, m, :], Act.Relu,
                        bias=pu_sb[:, m, 2, jb:jb + 1],
                        scale=pu_sb[:, m, 0, jb:jb + 1],
                    )
                else:
                    nc.vector.tensor_scalar(
                        k_t[:, m, :], rb_sb[:, m, :],
                        pu_sb[:, m, 0, jb:jb + 1], pu_sb[:, m, 2, jb:jb + 1],
                        Alu.mult, op1=Alu.add,
                    )
                    nc.vector.tensor_scalar(
                        k_t[:, m, :], k_t[:, m, :], 0.0, None, Alu.max,
                    )
            # mask: P = K min adjT (adjT broadcast across mechs) on DVE
            p_t = pp.tile([128, M, ROWS], BF16, tag="p")
            at = adjt_g[jb // 2][:, jb % 2, :]
            at_b = bass.AP(at.tensor, at.offset,
                           [list(at.ap[0]), [0, M], [1, ROWS]])
            nc.vector.tensor_tensor(p_t, k_t, at_b, Alu.min)
            # accumulate out^T[m] += h_aug[jb, m].T @ P[m]
            for m in range(M):
                nc.tensor.matmul(
                    acc[m],
                    lhsT=h_g[jb // 4][:, jb % 4, m, :],
                    rhs=p_t[:, m, :],
                    start=(jb == 0), stop=(jb == JB - 1),
                )
            # mech 2,3 u-terms: acc_m += hu_m[jb].T @ adjT[jb]
            for i, m in enumerate((2, 3)):
                hu_l = (hu_g[i][0][:, jb, :] if jb < 4
                        else hu_g[i][1][:, jb - 4, :])
                nc.tensor.matmul(
                    acc[m],
                    lhsT=hu_l,
                    rhs=at,
                    start=False, stop=(jb == JB - 1),
                )

        # ---- epilogue: transpose, normalize, elu, store ----
        # per 128-row chunk: 4 transposes into one PSUM tile, one strided
        # reciprocal over the 4 denominators, then ELU on the whole chunk.
        out_r = out_d.rearrange("(c p) f -> c p f", p=128)
        o65d = {}
        for cp in range(IT // 2):
            for m in range(M):
                o65 = fin.tile([65, 256], F32, tag="o65", name="o65")
                nc.scalar.activation(
                    o65, acc[m][:, cp * 256:(cp + 1) * 256], Act.Copy)
                o65d[(cp, m)] = o65
        for c in range(IT):
            pt_t = tpp.tile([128, M, 65], F32, tag="ptt")
            for m in range(M):
                o65 = o65d[(c // 2, m)]
                h = (c % 2) * 128
                nc.tensor.transpose(pt_t[:, m, :], o65[:, h:h + 128],
                                    ident[0:65, 0:65])
            rcp = fin.tile([128, M], F32, tag="rcp")
            pt_dn = pt_t[:, 0, 64:65]
            dn = bass.AP(pt_dn.tensor, pt_dn.offset,
                         [list(pt_dn.ap[0]), [65, M]])
            nc.vector.reciprocal(rcp, dn)
            xn = fin.tile([128, M, OUTS], F32, tag="xn")
            for m in range(M):
                nc.vector.tensor_scalar(
                    xn[:, m, :], pt_t[:, m, 0:OUTS], rcp[:, m:m + 1], None,
                    Alu.mult)
            ob = fin.tile([128, M * OUTS], F32, tag="ob")
            mn = fin.tile([128, M * OUTS], F32, tag="mn")
            nc.gpsimd.tensor_scalar(mn, xn, 0.0, None, Alu.min)
            eq = fin.tile([128, M * OUTS], F32, tag="eq")
            nc.scalar.activation(eq, mn, Act.Exp)
            nc.vector.scalar_tensor_tensor(
                ob, eq, -1.0, xn.rearrange("p m o -> p (m o)"),
                Alu.add, Alu.max,
            )
            nc.sync.dma_start(out_r[c], ob)


_CACHE = {}


def _build():
    if "nc" in _CACHE:
        return _CACHE["nc"]
    nc = bacc.Bacc("TRN2", target_bir_lowering=False, debug=False,
                   num_devices=NCORES)
    adjt_d = nc.dram_tensor("adjt", [128, JB, ROWS], BF16,
                            kind="ExternalInput").ap()
    h_d = nc.dram_tensor("h_aug", [128, JB, M, 65], BF16,
                         kind="ExternalInput").ap()
    hu_d = [nc.dram_tensor(f"hu{m}", [128, JB, 65], BF16,
                           kind="ExternalInput").ap() for m in (2, 3)]
    rb_d = nc.dram_tensor("rb", [128, M, ROWS], BF16,
                          kind="ExternalInput").ap()
    pu_d = nc.dram_tensor("pu", [128, M, 3, JB], F32,
                          kind="ExternalInput").ap()
    out_d = nc.dram_tensor("out", [ROWS, M * OUTS], F32,
                           kind="ExternalOutput").ap()
    with tile.TileContext(nc) as tc:
        _trace_kernel(tc, out_d, adjt_d, h_d, hu_d, rb_d, pu_d)
    nc.compile()
    _CACHE["nc"] = nc
    return nc


def host_prep(x, adj, W, a1, a2, Wc, bc):
    x = np.asarray(x, np.float32)
    pooled = x.mean(0)
    gb = (pooled @ np.asarray(Wc, np.float32) + np.asarray(bc, np.float32))
    gb = gb.reshape(2, M, OUTS)
    gamma, beta = gb[0], gb[1]
    h = np.einsum("ni,mio->mno", x, np.asarray(W, np.float32))
    h = gamma[:, None, :] * h + beta[:, None, :]          # [M, N, OUTS]
    e_src = np.einsum("mno,mo->mn", h, np.asarray(a1, np.float32))
    e_dst = np.einsum("mno,mo->mn", h, np.asarray(a2, np.float32))

    u = np.exp(e_dst)                    # [M, N]
    p = np.exp(LEAK * e_dst)
    r = np.exp((LEAK - 1.0) * e_src)
    c = K_MARGIN / np.maximum(u.max(axis=1),
                              r.max(axis=1) * p.max(axis=1))  # [M]
    u *= c[:, None]
    p *= c[:, None]

    # h_aug [128, JB, M, 65] lhsT tiles (ones column -> softmax denominator)
    h_aug = np.zeros((N, M, 65), np.float32)
    for m in range(M):
        h_aug[:, m, 0:OUTS] = h[m]
        h_aug[:, m, OUTS] = 1.0
    h_tiles = np.ascontiguousarray(
        h_aug.reshape(JB, 128, M, 65).transpose(1, 0, 2, 3)
    ).astype(ml_dtypes.bfloat16)

    # hu_m [128, JB, 65]: h_aug of mech m scaled by u_m per row j
    hu_tiles = {}
    for m in (2, 3):
        hum = h_aug[:, m, :] * u[m][:, None]
        hu_tiles[m] = np.ascontiguousarray(
            hum.reshape(JB, 128, 65).transpose(1, 0, 2)
        ).astype(ml_dtypes.bfloat16)

    # per-j-block scalar columns: [128, M, 3, JB] (p, u, -u)
    pu = np.empty((128, M, 3, JB), np.float32)
    for m in range(M):
        pu[:, m, 0, :] = p[m].reshape(JB, 128).T
        pu[:, m, 1, :] = u[m].reshape(JB, 128).T
        pu[:, m, 2, :] = -u[m].reshape(JB, 128).T

    adjb = np.asarray(adj, np.int32).astype(ml_dtypes.bfloat16)

    in_maps = []
    for cc in range(NCORES):
        sl = slice(cc * ROWS, (cc + 1) * ROWS)
        adjt = np.ascontiguousarray(
            adjb[sl].T.reshape(JB, 128, ROWS).transpose(1, 0, 2))
        rb = np.ascontiguousarray(
            np.broadcast_to(r[:, sl], (128, M, ROWS))
        ).astype(ml_dtypes.bfloat16)
        in_maps.append({
            "adjt": adjt,
            "h_aug": h_tiles,
            "hu2": hu_tiles[2],
            "hu3": hu_tiles[3],
            "rb": rb,
            "pu": pu,
        })
    return in_maps


def kernel(x, adj, W, a1, a2, Wc, bc):
    nc = _build()
    in_maps = host_prep(x, adj, W, a1, a2, Wc, bc)
    res = bass_utils.run_bass_kernel_spmd(
        nc, in_maps, core_ids=list(range(NCORES))
    )
    out = np.concatenate([res.results[c]["out"] for c in range(NCORES)], axis=0)
    return out.astype(np.float32)


# revision 24
# speedup vs baseline: 1.0105x; 1.0031x over previous
"""Trainium2 Bass kernel for nn_ConditionalAttentionLayer.

Row-sharded across 8 NeuronCores: core c computes output rows
[c*512, (c+1)*512).  Math identities used on device:

    exp(leaky_relu(s)) = max(exp(s), exp(0.2*s)),  s = e_src[i] + e_dst[j]

factors rank-1, and softmax is scale-invariant per column, so the v_i =
exp(e_src[i]) factor cancels:  the device builds

    P[j,i] = adjT[j,i] * max(u_j, r_i * p_j)

with u = exp(e_dst)*c, p = exp(0.2*e_dst)*c, r = exp(-0.8*e_src), c a
per-mech scale keeping every value < 1 so the adjacency mask is a plain
min().  Per j-block this costs one 4x-mode tensor_scalar (mult,max) per
mechanism on DVE plus one masking tensor_tensor split between DVE and
Pool.  Mechanism 3 instead uses max(u,rp) = u + relu(rp-u): ACT computes
relu(p*r - u) and the u*adjT term folds into an extra PE matmul against
host-precomputed hu3 = h_aug3*u3, spreading the elementwise work over
all four engines.  adj is transposed/cast to bf16 on the host so the
device streams it straight into SBUF tiles.
"""

import sys
from contextlib import ExitStack

import numpy as np
import ml_dtypes

sys.path.insert(0, "/opt/trn_rl_repo")

import concourse.bass as bass  # noqa: E402
import concourse.bacc as bacc  # noqa: E402
import concourse.tile as tile  # noqa: E402
import concourse.mybir as mybir  # noqa: E402
from concourse import bass_utils  # noqa: E402
from concourse.masks import make_identity  # noqa: E402

N = 4096
INS = 256
OUTS = 64
M = 4
NCORES = 8
ROWS = N // NCORES      # 512 output rows per core
JB = N // 128           # 32 j-blocks
IT = ROWS // 128        # 4 i-tiles per core
SD = 256                # mech-0 pass-1 columns on DVE; Pool takes the rest
POOL_P1 = True          # debug: route Pool pass-1 work to DVE when False
ACT_P1 = True           # debug: emulate ACT relu path on DVE when False
LEAK = 0.2
K_MARGIN = 0.95         # keep max(u, r*p) < 1 so min(x, adjT) masks

F32 = mybir.dt.float32
BF16 = mybir.dt.bfloat16
Alu = mybir.AluOpType
Act = mybir.ActivationFunctionType


def _trace_kernel(tc, out_d, adjt_d, h_d, hu_d, rb_d, pu_d):
    nc = tc.nc
    with ExitStack() as ctx:
        const = ctx.enter_context(tc.tile_pool(name="const", bufs=1))
        kp = ctx.enter_context(tc.tile_pool(name="kp", bufs=4))
        pp = ctx.enter_context(tc.tile_pool(name="pp", bufs=4))
        accp = ctx.enter_context(tc.tile_pool(name="acc", bufs=1, space="PSUM"))
        tpp = ctx.enter_context(tc.tile_pool(name="tp", bufs=3, space="PSUM"))
        fin = ctx.enter_context(tc.tile_pool(name="fin", bufs=8))

        # ---- persistent SBUF tensors ----
        pu_sb = const.tile([128, M, 3, JB], F32, tag="pu")
        rb_sb = const.tile([128, M, ROWS], BF16, tag="rb")
        h_g = [const.tile([128, 4, M, 65], BF16, tag=f"h{g}", name=f"h{g}")
               for g in range(8)]
        hu_g = [[const.tile([128, 4, 65], BF16, tag=f"hu{m}a", name=f"hu{m}a"),
                 const.tile([128, 28, 65], BF16, tag=f"hu{m}b", name=f"hu{m}b")]
                for m in (2, 3)]
        adjt_g = [const.tile([128, 2, ROWS], BF16, tag=f"adjt{g}",
                             name=f"adjt{g}") for g in range(16)]
        ident = const.tile([128, 128], F32, tag="ident")
        make_identity(nc, ident)
        actw = const.tile([128, 1], F32, tag="actw")
        nc.scalar.activation(actw, ident[:, 0:1], Act.Relu)
        warm = accp.tile([128, 128], F32, tag="warm", name="warm")
        for _ in range(25):
            nc.tensor.transpose(warm, ident, ident)

        # small params first, then chunks ordered so jb 0 unblocks fast.
        # adjt/h/hu chunk tiles are whole-tile DMA targets: partial-slice
        # writes into one big tile lose WAR/RAW deps in the tile framework.
        nc.sync.dma_start(rb_sb, rb_d)
        nc.sync.dma_start(pu_sb, pu_d)
        nc.sync.dma_start(adjt_g[0], adjt_d[:, 0:2, :])
        nc.sync.dma_start(h_g[0], h_d[:, 0:4, :, :])
        nc.sync.dma_start(hu_g[0][0], hu_d[0][:, 0:4, :])
        nc.sync.dma_start(hu_g[1][0], hu_d[1][:, 0:4, :])
        nc.sync.dma_start(adjt_g[1], adjt_d[:, 2:4, :])
        nc.sync.dma_start(hu_g[0][1], hu_d[0][:, 4:32, :])
        nc.sync.dma_start(hu_g[1][1], hu_d[1][:, 4:32, :])
        for g in range(2, 16):
            a, b = g * 2, (g + 1) * 2
            nc.sync.dma_start(adjt_g[g], adjt_d[:, a:b, :])
            if g % 2 == 1:
                gg = (g - 1) // 2
                nc.sync.dma_start(h_g[gg], h_d[:, gg * 4:(gg + 1) * 4, :, :])

        # ---- psum accumulators: one [65, ROWS] bank per mechanism ----
        acc = [accp.tile([65, ROWS], F32, tag=f"acc{m}", name=f"acc{m}")
               for m in range(M)]

        # ---- heavy loop ----
        for jb in range(JB):
            k_t = kp.tile([128, M, ROWS], BF16, tag="k")
            # mech 0: K = (r * p_j) max u_j, split DVE / Pool by columns
            nc.vector.tensor_scalar(
                k_t[:, 0, 0:SD], rb_sb[:, 0, 0:SD],
                pu_sb[:, 0, 0, jb:jb + 1], pu_sb[:, 0, 1, jb:jb + 1],
                Alu.mult, op1=Alu.max,
            )
            eng0 = nc.gpsimd if POOL_P1 else nc.vector
            eng0.tensor_scalar(
                k_t[:, 0, SD:ROWS], rb_sb[:, 0, SD:ROWS],
                pu_sb[:, 0, 0, jb:jb + 1], pu_sb[:, 0, 1, jb:jb + 1],
                Alu.mult, op1=Alu.max,
            )
            # mech 1 fully on Pool
            eng0.tensor_scalar(
                k_t[:, 1, :], rb_sb[:, 1, :],
                pu_sb[:, 1, 0, jb:jb + 1], pu_sb[:, 1, 1, jb:jb + 1],
                Alu.mult, op1=Alu.max,
            )
            # mechs 2,3 on ACT: K' = relu(r*p_j - u_j); u_j*adjT goes via PE
            for m in (2, 3):
                if ACT_P1:
                    nc.scalar.activation(
                        k_t[:, m, :], rb_sb[:, m, :], Act.Relu,
                        bias=pu_sb[:, m, 2, jb:jb + 1],
                        scale=pu_sb[:, m, 0, jb:jb + 1],
                    )
                else:
                    nc.vector.tensor_scalar(
                        k_t[:, m, :], rb_sb[:, m, :],
                        pu_sb[:, m, 0, jb:jb + 1], pu_sb[:, m, 2, jb:jb + 1],
                        Alu.mult, op1=Alu.add,
                    )
                    nc.vector.tensor_scalar(
                        k_t[:, m, :], k_t[:, m, :], 0.0, None, Alu.max,
                    )
            # mask: P = K min adjT (adjT broadcast across mechs) on DVE
            p_t = pp.tile([128, M, ROWS], BF16, tag="p")
            at = adjt_g[jb // 2][:, jb % 2, :]
            at_b = bass.AP(at.tensor, at.offset,
                           [list(at.ap[0]), [0, M], [1, ROWS]])
            nc.vector.tensor_tensor(p_t, k_t, at_b, Alu.min)
            # accumulate out^T[m] += h_aug[jb, m].T @ P[m]
            for m in range(M):
                nc.tensor.matmul(
                    acc[m],
                    lhsT=h_g[jb // 4][:, jb % 4, m, :],
                    rhs=p_t[:, m, :],
                    start=(jb == 0), stop=(jb == JB - 1),
                )
            # mech 2,3 u-terms: acc_m += hu_m[jb].T @ adjT[jb]
            for i, m in enumerate((2, 3)):
                hu_l = (hu_g[i][0][:, jb, :] if jb < 4
                        else hu_g[i][1][:, jb - 4, :])
                nc.tensor.matmul(
                    acc[m],
                    lhsT=hu_l,
                    rhs=at,
                    start=False, stop=(jb == JB - 1),
                )

        # ---- epilogue: transpose, normalize, elu, store ----
        # per 128-row chunk: 4 transposes into one PSUM tile, one strided
        # reciprocal over the 4 denominators, then ELU on the whole chunk.
        out_r = out_d.rearrange("(c p) f -> c p f", p=128)
        o65d = {}
        for cp in range(IT // 2):
            for m in range(M):
                o65 = fin.tile([65, 256], F32, tag="o65", name="o65")
                nc.scalar.activation(
                    o65, acc[m][:, cp * 256:(cp + 1) * 256], Act.Copy)
                o65d[(cp, m)] = o65
        for c in range(IT):
            pt_t = tpp.tile([128, M, 65], F32, tag="ptt")
            for m in range(M):
                o65 = o65d[(c // 2, m)]
                h = (c % 2) * 128
                nc.tensor.transpose(pt_t[:, m, :], o65[:, h:h + 128],
                                    ident[0:65, 0:65])
            rcp = fin.tile([128, M], F32, tag="rcp")
            pt_dn = pt_t[:, 0, 64:65]
            dn = bass.AP(pt_dn.tensor, pt_dn.offset,
                         [list(pt_dn.ap[0]), [65, M]])
            nc.vector.reciprocal(rcp, dn)
            xn = fin.tile([128, M, OUTS], F32, tag="xn")
            for m in range(M):
                nc.vector.tensor_scalar(
                    xn[:, m, :], pt_t[:, m, 0:OUTS], rcp[:, m:m + 1], None,
                    Alu.mult)
            ob = fin.tile([128, M * OUTS], F32, tag="ob")
            mn = fin.tile([128, M * OUTS], F32, tag="mn")
            nc.vector.tensor_scalar(mn, xn, 0.0, None, Alu.min)
            eq = fin.tile([128, M * OUTS], F32, tag="eq")
            nc.scalar.activation(eq, mn, Act.Exp)
            nc.vector.scalar_tensor_tensor(
                ob, eq, -1.0, xn.rearrange("p m o -> p (m o)"),
                Alu.add, Alu.max,
            )
            nc.sync.dma_start(out_r[c], ob)


_CACHE = {}


def _build():
    if "nc" in _CACHE:
        return _CACHE["nc"]
    nc = bacc.Bacc("TRN2", target_bir_lowering=False, debug=False,
                   num_devices=NCORES)
    adjt_d = nc.dram_tensor("adjt", [128, JB, ROWS], BF16,
                            kind="ExternalInput").ap()
    h_d = nc.dram_tensor("h_aug", [128, JB, M, 65], BF16,
                         kind="ExternalInput").ap()
    hu_d = [nc.dram_tensor(f"hu{m}", [128, JB, 65], BF16,
                           kind="ExternalInput").ap() for m in (2, 3)]
    rb_d = nc.dram_tensor("rb", [128, M, ROWS], BF16,
                          kind="ExternalInput").ap()
    pu_d = nc.dram_tensor("pu", [128, M, 3, JB], F32,
                          kind="ExternalInput").ap()
    out_d = nc.dram_tensor("out", [ROWS, M * OUTS], F32,
                           kind="ExternalOutput").ap()
    with tile.TileContext(nc) as tc:
        _trace_kernel(tc, out_d, adjt_d, h_d, hu_d, rb_d, pu_d)
    nc.compile()
    _CACHE["nc"] = nc
    return nc


def host_prep(x, adj, W, a1, a2, Wc, bc):
    x = np.asarray(x, np.float32)
    pooled = x.mean(0)
    gb = (pooled @ np.asarray(Wc, np.float32) + np.asarray(bc, np.float32))
    gb = gb.reshape(2, M, OUTS)
    gamma, beta = gb[0], gb[1]
    h = np.einsum("ni,mio->mno", x, np.asarray(W, np.float32))
    h = gamma[:, None, :] * h + beta[:, None, :]          # [M, N, OUTS]
    e_src = np.einsum("mno,mo->mn", h, np.asarray(a1, np.float32))
    e_dst = np.einsum("mno,mo->mn", h, np.asarray(a2, np.float32))

    u = np.exp(e_dst)                    # [M, N]
    p = np.exp(LEAK * e_dst)
    r = np.exp((LEAK - 1.0) * e_src)
    c = K_MARGIN / np.maximum(u.max(axis=1),
                              r.max(axis=1) * p.max(axis=1))  # [M]
    u *= c[:, None]
    p *= c[:, None]

    # h_aug [128, JB, M, 65] lhsT tiles (ones column -> softmax denominator)
    h_aug = np.zeros((N, M, 65), np.float32)
    for m in range(M):
        h_aug[:, m, 0:OUTS] = h[m]
        h_aug[:, m, OUTS] = 1.0
    h_tiles = np.ascontiguousarray(
        h_aug.reshape(JB, 128, M, 65).transpose(1, 0, 2, 3)
    ).astype(ml_dtypes.bfloat16)

    # hu_m [128, JB, 65]: h_aug of mech m scaled by u_m per row j
    hu_tiles = {}
    for m in (2, 3):
        hum = h_aug[:, m, :] * u[m][:, None]
        hu_tiles[m] = np.ascontiguousarray(
            hum.reshape(JB, 128, 65).transpose(1, 0, 2)
        ).astype(ml_dtypes.bfloat16)

    # per-j-block scalar columns: [128, M, 3, JB] (p, u, -u)
    pu = np.empty((128, M, 3, JB), np.float32)
    for m in range(M):
        pu[:, m, 0, :] = p[m].reshape(JB, 128).T
        pu[:, m, 1, :] = u[m].reshape(JB, 128).T
        pu[:, m, 2, :] = -u[m].reshape(JB, 128).T

    adjb = np.asarray(adj, np.int32).astype(ml_dtypes.bfloat16)

    in_maps = []
    for cc in range(NCORES):
        sl = slice(cc * ROWS, (cc + 1) * ROWS)
        adjt = np.ascontiguousarray(
            adjb[sl].T.reshape(JB, 128, ROWS).transpose(1, 0, 2))
        rb = np.ascontiguousarray(
            np.broadcast_to(r[:, sl], (128, M, ROWS))
        ).astype(ml_dtypes.bfloat16)
        in_maps.append({
            "adjt": adjt,
            "h_aug": h_tiles,
            "hu2": hu_tiles[2],
            "hu3": hu_tiles[3],
            "rb": rb,
            "pu": pu,
        })
    return in_maps


def kernel(x, adj, W, a1, a2, Wc, bc):
    nc = _build()
    in_maps = host_prep(x, adj, W, a1, a2, Wc, bc)
    res = bass_utils.run_bass_kernel_spmd(
        nc, in_maps, core_ids=list(range(NCORES))
    )
    out = np.concatenate([res.results[c]["out"] for c in range(NCORES)], axis=0)
    return out.astype(np.float32)
